# revision 1
# baseline (speedup 1.0000x reference)
"""Trainium2 Bass kernel for AdaptiveReLU segment-reduce.

Reference computation (per segment s over instance rows x[i] with batch_idx[i]==s):
    mn = min, mx = max, sums = sum, n = count
    bias = t*mx + (1-t)*mn            (t clamped to [0,1], per feature)
    relu_sum = sum(relu(x - bias))
    out[s,f] = W0*n + W1*mn + W2*mx + W3*relu_sum + W4*sums

Strategy: host-side sort + count-sorted packing so every segment lives on one
core with near-zero padding, then a fully local (collective-free) SPMD kernel
on 8 NeuronCores.

Packing layout (per core):
  - Segments are globally sorted by count (desc) and dealt into
    8 cores x NB blocks; block position b holds 256 segments per core, all
    padded to the same length L_b (equal across cores -> one SPMD graph).
  - Block SBUF tile: [128 partitions, L_b*128 columns], partition
    p = par*64 + f  (par in {0,1}, f = feature), column j*128 + g
    (j = row-within-segment, g = segment-group).  Segment identity is
    (g, par).  Padding rows replicate the segment's first row, so min/max
    are unaffected; sums are corrected with the known pad counts.
  - relu_sum is computed without materializing relu:
        sum(relu(y)) = 0.5*(sum|y| + sum(y)),  y = x - bias
    using tensor_reduce(apply_absolute_value=True).
"""

import os
import numpy as np

F = 64            # feature dim
G = 128           # segment-groups per block (stats tile columns)
SPB = 2 * G       # segments per block (2 parity lanes)
NCORES = 8
COMPUTE_DT = os.environ.get("KERNEL_DT", "f32")  # "f32" | "bf16"


def _pack(x, batch_idx, S):
    """Sort+pack inputs. Returns (in_maps, Ls, order) for unpacking."""
    N = x.shape[0]
    rps = SPB * NCORES                      # ranks per super-group
    NB = S // rps
    assert S % rps == 0, (S, rps)

    counts = np.bincount(batch_idx, minlength=S).astype(np.int64)
    order = np.argsort(-counts, kind="stable").astype(np.int64)
    sc = counts[order]
    assert sc[-1] >= 1, "empty segments unsupported"
    Ls = np.maximum(sc[::rps], 1).astype(np.int64)        # [NB]

    perm = np.argsort(batch_idx, kind="stable").astype(np.int64)
    seg_start = np.zeros(S + 1, np.int64)
    np.cumsum(counts, out=seg_start[1:])

    in_maps = [dict() for _ in range(NCORES)]
    W_total = int((Ls * G).sum())
    for c in range(NCORES):
        xcore = np.empty((128, W_total), np.float32)
        ncore = np.empty((128, G * NB), np.float32)
        col = 0
        for b in range(NB):
            L = int(Ls[b])
            ranks = rps * b + SPB * c + np.arange(SPB)
            segs = order[ranks]                            # [256] k=2g+par
            cnt = counts[segs]                             # [256]
            j = np.arange(L)[None, :]
            jeff = np.where(j < cnt[:, None], j, 0)        # replicate first row
            rows = perm[seg_start[segs][:, None] + jeff]   # [256, L]
            blk = x[rows]                                  # [256, L, 64]
            # (k=2g+par, j, f) -> (par, f, j, g) -> [128, L*G]
            blk = blk.reshape(G, 2, L, F).transpose(1, 3, 2, 0)
            xcore[:, col:col + L * G] = blk.reshape(128, L * G)
            cblk = cnt.reshape(G, 2).T.astype(np.float32)  # [2, G]
            ncore[:, b * G:(b + 1) * G] = np.broadcast_to(
                cblk[:, None, :], (2, F, G)).reshape(128, G)
            col += L * G
        in_maps[c]["xb"] = xcore
        in_maps[c]["ncnt"] = ncore
    return in_maps, Ls, order


def _build(Ls, Wvals):
    """Build the SPMD Bass graph. Returns compiled Bacc module."""
    import concourse.bass as bass
    import concourse.tile as tile
    from concourse import bacc, mybir

    f32 = mybir.dt.float32
    AX = mybir.AxisListType.X
    OP = mybir.AluOpType
    ACT = mybir.ActivationFunctionType

    NB = len(Ls)
    W_total = int(sum(int(L) * G for L in Ls))
    W0, W1, W2, W3, W4 = [float(v) for v in Wvals]

    nc = bacc.Bacc("TRN2", target_bir_lowering=False, debug=False,
                   num_devices=NCORES)
    xdr = nc.dram_tensor("xb", [128, W_total], f32, kind="ExternalInput").ap()
    ndr = nc.dram_tensor("ncnt", [128, G * NB], f32, kind="ExternalInput").ap()
    tdr = nc.dram_tensor("tpar", [128, 1], f32, kind="ExternalInput").ap()
    odr = nc.dram_tensor("out", [128, G * NB], f32, kind="ExternalOutput").ap()

    with tile.TileContext(nc) as tc, \
         tc.tile_pool(name="xpool", bufs=2) as xpool, \
         tc.tile_pool(name="ypool", bufs=2) as ypool, \
         tc.tile_pool(name="spool", bufs=3) as spool, \
         tc.tile_pool(name="cpool", bufs=1) as cpool:

        tpp = cpool.tile([128, 1], f32)
        nc.sync.dma_start(tpp[:], tdr)
        tcl = cpool.tile([128, 1], f32)
        nc.vector.tensor_scalar(tcl[:], tpp[:], 0.0, 1.0, OP.max, OP.min)
        onemt = cpool.tile([128, 1], f32)
        nc.vector.tensor_scalar(onemt[:], tcl[:], -1.0, 1.0, OP.mult, OP.add)
        nct = cpool.tile([128, G * NB], f32)
        nc.sync.dma_start(nct[:], ndr)

        col = 0
        for b in range(NB):
            L = int(Ls[b])
            Wb = L * G
            xt = xpool.tile([128, Wb], f32, tag="xt")
            nc.sync.dma_start(xt[:], xdr[:, col:col + Wb])
            x3 = xt[:].rearrange("p (j g) -> p g j", g=G)

            mn = spool.tile([128, G], f32, tag="mn")
            nc.vector.tensor_reduce(mn[:], x3, axis=AX, op=OP.min)
            mx = spool.tile([128, G], f32, tag="mx")
            nc.vector.tensor_reduce(mx[:], x3, axis=AX, op=OP.max)
            sxp = spool.tile([128, G], f32, tag="sxp")
            nc.vector.tensor_reduce(sxp[:], x3, axis=AX, op=OP.add)

            biasA = spool.tile([128, G], f32, tag="biasA")
            nc.vector.tensor_scalar_mul(biasA[:], mx[:], tcl[:])
            bias = spool.tile([128, G], f32, tag="bias")
            nc.vector.scalar_tensor_tensor(
                bias[:], mn[:], onemt[:], biasA[:], OP.mult, OP.add)

            yt = ypool.tile([128, Wb], f32, tag="yt")
            y3 = yt[:].rearrange("p (j g) -> p g j", g=G)
            bias_b = bias[:].unsqueeze(2).broadcast_to([128, G, L])
            nc.vector.tensor_tensor(y3, x3, bias_b, op=OP.subtract)

            sap = spool.tile([128, G], f32, tag="sap")
            nc.vector.tensor_reduce(sap[:], y3, axis=AX, op=OP.add,
                                    apply_absolute_value=True)

            nblk = nct[:, b * G:(b + 1) * G]
            padn = spool.tile([128, G], f32, tag="padn")
            nc.vector.tensor_scalar(padn[:], nblk, -1.0, float(L),
                                    OP.mult, OP.add)

            yfabs = spool.tile([128, G], f32, tag="yfabs")
            nc.scalar.activation(yfabs[:], yt[:, 0:G], ACT.Abs)

            c1 = spool.tile([128, G], f32, tag="c1")
            nc.vector.tensor_mul(c1[:], padn[:], xt[:, 0:G])
            sx = spool.tile([128, G], f32, tag="sx")
            nc.vector.tensor_sub(sx[:], sxp[:], c1[:])

            c2 = spool.tile([128, G], f32, tag="c2")
            nc.vector.tensor_mul(c2[:], padn[:], yfabs[:])
            sa = spool.tile([128, G], f32, tag="sa")
            nc.vector.tensor_sub(sa[:], sap[:], c2[:])

            nb2 = spool.tile([128, G], f32, tag="nb2")
            nc.vector.tensor_mul(nb2[:], nblk, bias[:])
            sy = spool.tile([128, G], f32, tag="sy")
            nc.vector.tensor_sub(sy[:], sx[:], nb2[:])

            rs = spool.tile([128, G], f32, tag="rs")
            nc.vector.tensor_add(rs[:], sa[:], sy[:])   # rs = 2*relu_sum

            acc0 = spool.tile([128, G], f32, tag="acc0")
            nc.vector.tensor_scalar_mul(acc0[:], nblk, W0)
            acc1 = spool.tile([128, G], f32, tag="acc1")
            nc.vector.scalar_tensor_tensor(
                acc1[:], mn[:], W1, acc0[:], OP.mult, OP.add)
            acc2 = spool.tile([128, G], f32, tag="acc2")
            nc.vector.scalar_tensor_tensor(
                acc2[:], mx[:], W2, acc1[:], OP.mult, OP.add)
            acc3 = spool.tile([128, G], f32, tag="acc3")
            nc.vector.scalar_tensor_tensor(
                acc3[:], rs[:], 0.5 * W3, acc2[:], OP.mult, OP.add)
            acc4 = spool.tile([128, G], f32, tag="acc4")
            nc.vector.scalar_tensor_tensor(
                acc4[:], sx[:], W4, acc3[:], OP.mult, OP.add)

            nc.sync.dma_start(odr[:, b * G:(b + 1) * G], acc4[:])
            col += Wb

    nc.compile()
    return nc


LAST_EXEC_NS = None
LAST_RESULTS = None


def kernel(x, batch_idx, max_index, t, W):
    global LAST_EXEC_NS, LAST_RESULTS
    x = np.ascontiguousarray(np.asarray(x, dtype=np.float32))
    batch_idx_in = np.asarray(batch_idx)
    bidx = batch_idx_in.astype(np.int64)
    S = int(max_index)
    t_np = np.asarray(t, dtype=np.float32).reshape(F)
    W_np = np.asarray(W, dtype=np.float32).reshape(-1)
    assert x.shape[1] == F and W_np.shape[0] == 5

    in_maps, Ls, order = _pack(x, bidx, S)
    NB = len(Ls)
    tpar = np.tile(t_np, 2).reshape(128, 1).astype(np.float32)
    for m in in_maps:
        m["tpar"] = tpar

    nc = _build(Ls, W_np)

    if os.environ.get("KERNEL_SIM", "0") == "1":
        from concourse.bass_interp import CoreSim
        outs = []
        for c in range(NCORES):
            sim = CoreSim(nc, trace=False)
            for k, v in in_maps[c].items():
                sim.tensor(k)[:] = v
            sim.simulate(check_with_hw=False)
            outs.append(np.array(sim.tensor("out")))
        results = [{"out": o} for o in outs]
        LAST_EXEC_NS = None
    else:
        from concourse import bass_utils
        trace = os.environ.get("KERNEL_TRACE", "0") == "1"
        tmpdir = os.environ.get("KERNEL_TRACE_DIR") or None
        res = bass_utils.run_bass_kernel_spmd(
            nc, in_maps, core_ids=list(range(NCORES)),
            trace=trace, tmpdir=tmpdir)
        results = res.results
        LAST_EXEC_NS = res.exec_time_ns
        LAST_RESULTS = res

    # Unpack: out_dev [128, G*NB] -> [S, F] in original segment order
    rps = SPB * NCORES
    out_full = np.empty((S, F), np.float32)
    for c in range(NCORES):
        od = np.asarray(results[c]["out"])              # [128, G*NB]
        v = od.reshape(2, F, NB, G).transpose(2, 3, 0, 1)   # [NB, G, 2, F]
        v = v.reshape(NB * SPB, F)                      # rank-chunk order
        ranks = (rps * np.arange(NB)[:, None] + SPB * c
                 + np.arange(SPB)[None, :]).ravel()
        out_full[order[ranks]] = v
    return out_full


# revision 6
# speedup vs baseline: 2.0853x; 2.0853x over previous
"""Trainium2 Bass kernel for AdaptiveReLU segment-reduce.

Reference computation (per segment s over instance rows x[i] with batch_idx[i]==s):
    mn = min, mx = max, sums = sum, n = count
    bias = t*mx + (1-t)*mn            (t clamped to [0,1], per feature)
    relu_sum = sum(relu(x - bias))
    out[s,f] = W0*n + W1*mn + W2*mx + W3*relu_sum + W4*sums

Strategy: host-side sort + count-sorted packing so every segment lives on one
core with ~2.6% padding, then a fully local (collective-free) SPMD kernel on
8 NeuronCores.

Packing layout (per core):
  - Segments are globally sorted by count (desc) and dealt into 8 cores x NB
    block positions; block position b holds 256 segments per core, all padded
    to the same length L_b (equal across cores -> one SPMD graph).
  - Block SBUF tile: [128 partitions, L_b*128 columns] bf16, partition
    p = par*64 + f  (par in {0,1}, f = feature), column j*128 + g
    (j = row-within-segment, g = segment-group).  Segment identity is
    (g, par).  Padding rows replicate the segment's first row, so min/max
    are unaffected; sums are corrected with the known pad counts.
  - Reductions run as pairwise-halving trees of tensor_tensor ops (bf16
    2x_1p mode, 2 elem/cycle) instead of tensor_reduce (1x only).
    min/max trees on bf16 are exact; sum trees add ~0.2% noise, well under
    the 2e-2 gate.  relu runs on the otherwise-idle ScalarE.
"""

import os
import numpy as np
import ml_dtypes

F = 64            # feature dim
G = 128           # segment-groups per block (stats tile columns)
SPB = 2 * G       # segments per block (2 parity lanes)
NCORES = 8

BF16 = ml_dtypes.bfloat16


def _pack(x, batch_idx, S, Wvals):
    """Sort+pack inputs. Returns (in_maps, Ls, order)."""
    rps = SPB * NCORES                      # ranks per super-group
    NB = S // rps
    assert S % rps == 0, (S, rps)

    counts = np.bincount(batch_idx, minlength=S).astype(np.int64)
    order = np.argsort(-counts, kind="stable").astype(np.int64)
    sc = counts[order]
    assert sc[-1] >= 1, "empty segments unsupported"
    Ls = np.maximum(sc[::rps], 1).astype(np.int64)        # [NB]

    perm = np.argsort(batch_idx, kind="stable").astype(np.int64)
    seg_start = np.zeros(S + 1, np.int64)
    np.cumsum(counts, out=seg_start[1:])

    W0 = float(Wvals[0])
    in_maps = [dict() for _ in range(NCORES)]
    W_total = int((Ls * G).sum())
    xbf = x.astype(BF16)
    for c in range(NCORES):
        xcore = np.empty((128, W_total), BF16)
        aplane = np.empty((128, G * NB), np.float32)   # W0 * n
        pplane = np.empty((128, G * NB), BF16)         # pad count L_b - n
        col = 0
        for b in range(NB):
            L = int(Ls[b])
            ranks = rps * b + SPB * c + np.arange(SPB)
            segs = order[ranks]                            # [256] k=2g+par
            cnt = counts[segs]                             # [256]
            j = np.arange(L)[None, :]
            jeff = np.where(j < cnt[:, None], j, 0)        # replicate first row
            rows = perm[seg_start[segs][:, None] + jeff]   # [256, L]
            blk = xbf[rows]                                # [256, L, 64]
            # (k=2g+par, j, f) -> (par, f, j, g) -> [128, L*G]
            blk = blk.reshape(G, 2, L, F).transpose(1, 3, 2, 0)
            xcore[:, col:col + L * G] = blk.reshape(128, L * G)
            cblk = cnt.reshape(G, 2).T                     # [2, G]
            aplane[:, b * G:(b + 1) * G] = np.broadcast_to(
                (W0 * cblk)[:, None, :], (2, F, G)).reshape(128, G)
            pplane[:, b * G:(b + 1) * G] = np.broadcast_to(
                (float(L) - cblk)[:, None, :], (2, F, G)).reshape(128, G)
            col += L * G
        in_maps[c]["xb"] = xcore
        in_maps[c]["apl"] = aplane
        in_maps[c]["ppl"] = pplane
    return in_maps, Ls, order


def _tree(nc, pool, src_ap, L, dst_ap, op, tag, bf16):
    """Pairwise-halving reduction tree over j (column-groups of G).

    src_ap: flat [128, L*G] bf16 AP.  dst_ap: [128, G] AP (any dtype),
    written by the final level.
    """
    assert L >= 2
    cur = src_ap
    Lc = L
    lvl = 0
    while Lc > 1:
        h = Lc // 2
        odd = Lc % 2 == 1
        if h == 1:
            nxt = dst_ap          # final level writes the stats plane
        else:
            t = pool.tile([128, h * G], bf16, tag=f"tr{lvl}")
            nxt = t[:]
        nc.vector.tensor_tensor(
            nxt[:, 0:h * G], cur[:, 0:h * G], cur[:, h * G:2 * h * G], op=op)
        if odd:
            nc.vector.tensor_tensor(
                nxt[:, 0:G], nxt[:, 0:G], cur[:, 2 * h * G:Lc * G], op=op)
        cur = nxt
        Lc = h
        lvl += 1


LAST_EXEC_NS = None
LAST_RESULTS = None


def kernel(x, batch_idx, max_index, t, W):
    global LAST_EXEC_NS, LAST_RESULTS
    x = np.ascontiguousarray(np.asarray(x, dtype=np.float32))
    bidx = np.asarray(batch_idx).astype(np.int64)
    S = int(max_index)
    t_np = np.asarray(t, dtype=np.float32).reshape(F)
    W_np = np.asarray(W, dtype=np.float32).reshape(-1)
    assert x.shape[1] == F and W_np.shape[0] == 5

    in_maps, Ls, order = _pack(x, bidx, S, W_np)
    NB = len(Ls)
    tpar = np.tile(t_np, 2).reshape(128, 1).astype(np.float32)
    for m in in_maps:
        m["tpar"] = tpar

    nc = _build(Ls, W_np)

    if os.environ.get("KERNEL_SIM", "0") == "1":
        from concourse.bass_interp import CoreSim
        outs = []
        for c in range(NCORES):
            sim = CoreSim(nc, trace=False)
            for k, v in in_maps[c].items():
                sim.tensor(k)[:] = v
            sim.simulate(check_with_hw=False)
            outs.append(np.array(sim.tensor("out")))
        results = [{"out": o} for o in outs]
        LAST_EXEC_NS = None
    else:
        from concourse import bass_utils
        trace = os.environ.get("KERNEL_TRACE", "0") == "1"
        tmpdir = os.environ.get("KERNEL_TRACE_DIR") or None
        res = bass_utils.run_bass_kernel_spmd(
            nc, in_maps, core_ids=list(range(NCORES)),
            trace=trace, tmpdir=tmpdir)
        results = res.results
        LAST_EXEC_NS = res.exec_time_ns
        LAST_RESULTS = res

    # Unpack: out_dev [128, G*NB] -> [S, F] in original segment order
    rps = SPB * NCORES
    out_full = np.empty((S, F), np.float32)
    for c in range(NCORES):
        od = np.asarray(results[c]["out"])              # [128, G*NB]
        v = od.reshape(2, F, NB, G).transpose(2, 3, 0, 1)   # [NB, G, 2, F]
        v = v.reshape(NB * SPB, F)                      # rank-chunk order
        ranks = (rps * np.arange(NB)[:, None] + SPB * c
                 + np.arange(SPB)[None, :]).ravel()
        out_full[order[ranks]] = v
    return out_full


def _build(Ls, Wvals):
    """Build the SPMD Bass graph. Returns compiled Bacc module."""
    import concourse.tile as tile
    from concourse import bacc, mybir

    f32 = mybir.dt.float32
    bf16 = mybir.dt.bfloat16
    OP = mybir.AluOpType
    ACT = mybir.ActivationFunctionType

    NB = len(Ls)
    SB = G * NB
    W_total = int(sum(int(L) * G for L in Ls))
    W0, W1, W2, W3, W4 = [float(v) for v in Wvals]

    nc = bacc.Bacc("TRN2", target_bir_lowering=False, debug=False,
                   num_devices=NCORES)
    xdr = nc.dram_tensor("xb", [128, W_total], bf16, kind="ExternalInput").ap()
    adr = nc.dram_tensor("apl", [128, SB], f32, kind="ExternalInput").ap()
    pdr = nc.dram_tensor("ppl", [128, SB], bf16, kind="ExternalInput").ap()
    tdr = nc.dram_tensor("tpar", [128, 1], f32, kind="ExternalInput").ap()
    odr = nc.dram_tensor("out", [128, SB], f32, kind="ExternalOutput").ap()

    with tile.TileContext(nc) as tc, \
         tc.tile_pool(name="xpool", bufs=2) as xpool, \
         tc.tile_pool(name="ypool", bufs=2) as ypool, \
         tc.tile_pool(name="tpool", bufs=3) as tpool, \
         tc.tile_pool(name="bpool", bufs=2) as bpool, \
         tc.tile_pool(name="cpool", bufs=1) as cpool:

        tpp = cpool.tile([128, 1], f32)
        nc.sync.dma_start(tpp[:], tdr)
        tcl = cpool.tile([128, 1], f32)
        nc.vector.tensor_scalar(tcl[:], tpp[:], 0.0, 1.0, OP.max, OP.min)
        onemt = cpool.tile([128, 1], f32)
        nc.vector.tensor_scalar(onemt[:], tcl[:], -1.0, 1.0, OP.mult, OP.add)

        apl = cpool.tile([128, SB], f32)
        nc.sync.dma_start(apl[:], adr)
        ppl = cpool.tile([128, SB], bf16)
        nc.sync.dma_start(ppl[:], pdr)

        # persistent stats planes (min/max are exact in bf16)
        mnall = cpool.tile([128, SB], bf16)
        mxall = cpool.tile([128, SB], bf16)
        sxall = cpool.tile([128, SB], f32)
        srall = cpool.tile([128, SB], f32)

        col = 0
        for b in range(NB):
            L = int(Ls[b])
            Wb = L * G
            sl = slice(b * G, (b + 1) * G)
            xt = xpool.tile([128, Wb], bf16, tag="xt")
            nc.sync.dma_start(xt[:], xdr[:, col:col + Wb])

            _tree(nc, tpool, xt[:], L, mnall[:, sl], OP.min, "mn", bf16)
            _tree(nc, tpool, xt[:], L, mxall[:, sl], OP.max, "mx", bf16)
            _tree(nc, tpool, xt[:], L, sxall[:, sl], OP.add, "sx", bf16)

            # bias (bf16): bias = t*mx + (1-t)*mn  on this block's stats
            biasA = bpool.tile([128, G], bf16, tag="biasA")
            nc.vector.tensor_scalar_mul(biasA[:], mxall[:, sl], tcl[:])
            bias = bpool.tile([128, G], bf16, tag="bias")
            nc.vector.scalar_tensor_tensor(
                bias[:], mnall[:, sl], onemt[:], biasA[:], OP.mult, OP.add)

            # y = x - bias  (iterate [p, j, g]; g contiguous for 2x mode)
            yt = ypool.tile([128, Wb], bf16, tag="yt")
            xjg = xt[:].rearrange("p (j g) -> p j g", g=G)
            yjg = yt[:].rearrange("p (j g) -> p j g", g=G)
            bias_b = bias[:].unsqueeze(1).broadcast_to([128, L, G])
            nc.vector.tensor_tensor(yjg, xjg, bias_b, op=OP.subtract)

            # r = relu(y) in place on ScalarE (idle engine)
            nc.scalar.activation(yt[:], yt[:], ACT.Relu)

            _tree(nc, tpool, yt[:], L, srall[:, sl], OP.add, "sr", bf16)

            # pad corrections: sx -= (L-n)*x0 ; sr -= (L-n)*r0
            cx = bpool.tile([128, G], bf16, tag="cx")
            nc.vector.tensor_mul(cx[:], ppl[:, sl], xt[:, 0:G])
            nc.vector.tensor_sub(sxall[:, sl], sxall[:, sl], cx[:])
            cr = bpool.tile([128, G], bf16, tag="cr")
            nc.vector.tensor_mul(cr[:], ppl[:, sl], yt[:, 0:G])
            nc.vector.tensor_sub(srall[:, sl], srall[:, sl], cr[:])
            col += Wb

        # final combine on full [128, SB] planes (f32)
        tmpA = cpool.tile([128, SB], f32)
        nc.vector.scalar_tensor_tensor(
            tmpA[:], mnall[:], W1, apl[:], OP.mult, OP.add)
        nc.vector.scalar_tensor_tensor(
            apl[:], mxall[:], W2, tmpA[:], OP.mult, OP.add)
        nc.vector.scalar_tensor_tensor(
            tmpA[:], srall[:], W3, apl[:], OP.mult, OP.add)
        nc.vector.scalar_tensor_tensor(
            apl[:], sxall[:], W4, tmpA[:], OP.mult, OP.add)

        nc.sync.dma_start(odr[:], apl[:])

    nc.compile()
    return nc


# revision 8
# speedup vs baseline: 2.5368x; 1.2165x over previous
"""Trainium2 Bass kernel for AdaptiveReLU segment-reduce.

Reference computation (per segment s over instance rows x[i] with batch_idx[i]==s):
    mn = min, mx = max, sums = sum, n = count
    bias = t*mx + (1-t)*mn            (t clamped to [0,1], per feature)
    relu_sum = sum(relu(x - bias))
    out[s,f] = W0*n + W1*mn + W2*mx + W3*relu_sum + W4*sums

Strategy: host-side sort + count-sorted packing so every segment lives on one
core with ~2.6% padding, then a fully local (collective-free) SPMD kernel on
8 NeuronCores.

Packing layout (per core):
  - Segments are globally sorted by count (desc) and dealt into 8 cores x NB
    block positions; block position b holds 256 segments per core, all padded
    to the same length L_b (equal across cores -> one SPMD graph).
  - Block SBUF tile: [128 partitions, L_b*128 columns] bf16, partition
    p = par*64 + f  (par in {0,1}, f = feature), column j*128 + g
    (j = row-within-segment, g = segment-group).  Segment identity is
    (g, par).  Padding rows replicate the segment's first row, so min/max
    are unaffected; sums are corrected with the known pad counts.
  - Reductions run as pairwise-halving trees of tensor_tensor ops (bf16
    2x_1p mode, 2 elem/cycle) instead of tensor_reduce (1x only).
    min/max trees on bf16 are exact; sum trees add ~0.2% noise, well under
    the 2e-2 gate.  relu runs on the otherwise-idle ScalarE.
"""

import os
import numpy as np
import ml_dtypes

F = 64            # feature dim
G = 128           # segment-groups per block (stats tile columns)
SPB = 2 * G       # segments per block (2 parity lanes)
NCORES = 8

BF16 = ml_dtypes.bfloat16


def _pack(x, batch_idx, S, Wvals):
    """Sort+pack inputs. Returns (in_maps, Ls, order)."""
    rps = SPB * NCORES                      # ranks per super-group
    NB = S // rps
    assert S % rps == 0, (S, rps)

    counts = np.bincount(batch_idx, minlength=S).astype(np.int64)
    order = np.argsort(-counts, kind="stable").astype(np.int64)
    sc = counts[order]
    assert sc[-1] >= 1, "empty segments unsupported"
    Ls = np.maximum(sc[::rps], 1).astype(np.int64)        # [NB]

    perm = np.argsort(batch_idx, kind="stable").astype(np.int64)
    seg_start = np.zeros(S + 1, np.int64)
    np.cumsum(counts, out=seg_start[1:])

    W0 = float(Wvals[0])
    in_maps = [dict() for _ in range(NCORES)]
    W_total = int((Ls * G).sum())
    xbf = x.astype(BF16)
    for c in range(NCORES):
        xcore = np.empty((128, W_total), BF16)
        aplane = np.empty((128, G * NB), np.float32)   # W0 * n
        pplane = np.empty((128, G * NB), BF16)         # pad count L_b - n
        nplane = np.empty((128, G * NB), BF16)         # count n
        col = 0
        for b in range(NB):
            L = int(Ls[b])
            ranks = rps * b + SPB * c + np.arange(SPB)
            segs = order[ranks]                            # [256] k=2g+par
            cnt = counts[segs]                             # [256]
            j = np.arange(L)[None, :]
            jeff = np.where(j < cnt[:, None], j, 0)        # replicate first row
            rows = perm[seg_start[segs][:, None] + jeff]   # [256, L]
            blk = xbf[rows]                                # [256, L, 64]
            # (k=2g+par, j, f) -> (par, f, j, g) -> [128, L*G]
            blk = blk.reshape(G, 2, L, F).transpose(1, 3, 2, 0)
            xcore[:, col:col + L * G] = blk.reshape(128, L * G)
            cblk = cnt.reshape(G, 2).T                     # [2, G]
            aplane[:, b * G:(b + 1) * G] = np.broadcast_to(
                (W0 * cblk)[:, None, :], (2, F, G)).reshape(128, G)
            pplane[:, b * G:(b + 1) * G] = np.broadcast_to(
                (float(L) - cblk)[:, None, :], (2, F, G)).reshape(128, G)
            nplane[:, b * G:(b + 1) * G] = np.broadcast_to(
                cblk[:, None, :], (2, F, G)).reshape(128, G)
            col += L * G
        in_maps[c]["xb"] = xcore
        in_maps[c]["apl"] = aplane
        in_maps[c]["ppl"] = pplane
        in_maps[c]["npl"] = nplane
    return in_maps, Ls, order


def _tree(nc, pool, src_ap, L, dst_ap, op, tag, bf16):
    """Pairwise-halving reduction tree over j (column-groups of G).

    src_ap: flat [128, L*G] bf16 AP.  dst_ap: [128, G] AP (any dtype),
    written by the final level.
    """
    assert L >= 2
    cur = src_ap
    Lc = L
    lvl = 0
    while Lc > 1:
        h = Lc // 2
        odd = Lc % 2 == 1
        if h == 1:
            nxt = dst_ap          # final level writes the stats plane
        else:
            t = pool.tile([128, h * G], bf16, tag=f"tr{lvl}")
            nxt = t[:]
        nc.vector.tensor_tensor(
            nxt[:, 0:h * G], cur[:, 0:h * G], cur[:, h * G:2 * h * G], op=op)
        if odd:
            nc.vector.tensor_tensor(
                nxt[:, 0:G], nxt[:, 0:G], cur[:, 2 * h * G:Lc * G], op=op)
        cur = nxt
        Lc = h
        lvl += 1


LAST_EXEC_NS = None
LAST_RESULTS = None


def kernel(x, batch_idx, max_index, t, W):
    global LAST_EXEC_NS, LAST_RESULTS
    x = np.ascontiguousarray(np.asarray(x, dtype=np.float32))
    bidx = np.asarray(batch_idx).astype(np.int64)
    S = int(max_index)
    t_np = np.asarray(t, dtype=np.float32).reshape(F)
    W_np = np.asarray(W, dtype=np.float32).reshape(-1)
    assert x.shape[1] == F and W_np.shape[0] == 5

    in_maps, Ls, order = _pack(x, bidx, S, W_np)
    NB = len(Ls)
    tpar = np.tile(t_np, 2).reshape(128, 1).astype(np.float32)
    for m in in_maps:
        m["tpar"] = tpar

    nc = _build(Ls, W_np)

    if os.environ.get("KERNEL_SIM", "0") == "1":
        from concourse.bass_interp import CoreSim
        outs = []
        for c in range(NCORES):
            sim = CoreSim(nc, trace=False)
            for k, v in in_maps[c].items():
                sim.tensor(k)[:] = v
            sim.simulate(check_with_hw=False)
            outs.append(np.array(sim.tensor("out")))
        results = [{"out": o} for o in outs]
        LAST_EXEC_NS = None
    else:
        from concourse import bass_utils
        trace = os.environ.get("KERNEL_TRACE", "0") == "1"
        tmpdir = os.environ.get("KERNEL_TRACE_DIR") or None
        res = bass_utils.run_bass_kernel_spmd(
            nc, in_maps, core_ids=list(range(NCORES)),
            trace=trace, tmpdir=tmpdir)
        results = res.results
        LAST_EXEC_NS = res.exec_time_ns
        LAST_RESULTS = res

    # Unpack: out_dev [128, G*NB] -> [S, F] in original segment order
    rps = SPB * NCORES
    out_full = np.empty((S, F), np.float32)
    for c in range(NCORES):
        od = np.asarray(results[c]["out"])              # [128, G*NB]
        v = od.reshape(2, F, NB, G).transpose(2, 3, 0, 1)   # [NB, G, 2, F]
        v = v.reshape(NB * SPB, F)                      # rank-chunk order
        ranks = (rps * np.arange(NB)[:, None] + SPB * c
                 + np.arange(SPB)[None, :]).ravel()
        out_full[order[ranks]] = v
    return out_full


def _build(Ls, Wvals):
    """Build the SPMD Bass graph. Returns compiled Bacc module."""
    import concourse.tile as tile
    from concourse import bacc, mybir

    f32 = mybir.dt.float32
    bf16 = mybir.dt.bfloat16
    OP = mybir.AluOpType
    ACT = mybir.ActivationFunctionType

    NB = len(Ls)
    SB = G * NB
    W_total = int(sum(int(L) * G for L in Ls))
    W0, W1, W2, W3, W4 = [float(v) for v in Wvals]

    nc = bacc.Bacc("TRN2", target_bir_lowering=False, debug=False,
                   num_devices=NCORES)
    xdr = nc.dram_tensor("xb", [128, W_total], bf16, kind="ExternalInput").ap()
    adr = nc.dram_tensor("apl", [128, SB], f32, kind="ExternalInput").ap()
    pdr = nc.dram_tensor("ppl", [128, SB], bf16, kind="ExternalInput").ap()
    ndr = nc.dram_tensor("npl", [128, SB], bf16, kind="ExternalInput").ap()
    tdr = nc.dram_tensor("tpar", [128, 1], f32, kind="ExternalInput").ap()
    odr = nc.dram_tensor("out", [128, SB], f32, kind="ExternalOutput").ap()

    with tile.TileContext(nc) as tc, \
         tc.tile_pool(name="xpool", bufs=2) as xpool, \
         tc.tile_pool(name="ypool", bufs=2) as ypool, \
         tc.tile_pool(name="tpool", bufs=2) as tpool, \
         tc.tile_pool(name="bpool", bufs=2) as bpool, \
         tc.tile_pool(name="cpool", bufs=1) as cpool:

        tpp = cpool.tile([128, 1], f32)
        nc.sync.dma_start(tpp[:], tdr)
        tcl = cpool.tile([128, 1], f32)
        nc.vector.tensor_scalar(tcl[:], tpp[:], 0.0, 1.0, OP.max, OP.min)
        onemt = cpool.tile([128, 1], f32)
        nc.vector.tensor_scalar(onemt[:], tcl[:], -1.0, 1.0, OP.mult, OP.add)

        apl = cpool.tile([128, SB], f32)
        nc.sync.dma_start(apl[:], adr)
        ppl = cpool.tile([128, SB], bf16)
        nc.sync.dma_start(ppl[:], pdr)
        npl = cpool.tile([128, SB], bf16)
        nc.sync.dma_start(npl[:], ndr)

        # persistent stats planes (min/max are exact in bf16)
        mnall = cpool.tile([128, SB], bf16)
        mxall = cpool.tile([128, SB], bf16)
        sxall = cpool.tile([128, SB], f32)
        srall = cpool.tile([128, SB], f32)

        col = 0
        for b in range(NB):
            L = int(Ls[b])
            Wb = L * G
            sl = slice(b * G, (b + 1) * G)
            xt = xpool.tile([128, Wb], bf16, tag="xt")
            nc.sync.dma_start(xt[:], xdr[:, col:col + Wb])

            _tree(nc, tpool, xt[:], L, mnall[:, sl], OP.min, "mn", bf16)
            _tree(nc, tpool, xt[:], L, mxall[:, sl], OP.max, "mx", bf16)
            _tree(nc, tpool, xt[:], L, sxall[:, sl], OP.add, "sx", bf16)

            # bias (bf16): bias = t*mx + (1-t)*mn  on this block's stats
            # biasA = t*mx on ScalarE (idle engine; per-partition scale AP)
            biasA = bpool.tile([128, G], bf16, tag="biasA")
            nc.vector.tensor_scalar_mul(biasA[:], mxall[:, sl], tcl[:])
            bias = bpool.tile([128, G], bf16, tag="bias")
            nc.vector.scalar_tensor_tensor(
                bias[:], mnall[:, sl], onemt[:], biasA[:], OP.mult, OP.add)

            # max-trick: relu_sum = sum(max(x, bias)) - n*bias
            # mt = max(x, bias)  (iterate [p, j, g]; g contiguous for 2x mode)
            mt = ypool.tile([128, Wb], bf16, tag="mt")
            xjg = xt[:].rearrange("p (j g) -> p j g", g=G)
            mjg = mt[:].rearrange("p (j g) -> p j g", g=G)
            bias_b = bias[:].unsqueeze(1).broadcast_to([128, L, G])
            nc.vector.tensor_tensor(mjg, xjg, bias_b, op=OP.max)

            _tree(nc, tpool, mt[:], L, srall[:, sl], OP.add, "sr", bf16)

            # pad corrections: sx -= (L-n)*x0 ; sm -= (L-n)*mt0 + n*bias
            cx = bpool.tile([128, G], bf16, tag="cx")
            nc.vector.tensor_mul(cx[:], ppl[:, sl], xt[:, 0:G])
            nc.vector.tensor_sub(sxall[:, sl], sxall[:, sl], cx[:])
            cr = bpool.tile([128, G], bf16, tag="cr")
            nc.vector.tensor_mul(cr[:], ppl[:, sl], mt[:, 0:G])
            nc.vector.tensor_sub(srall[:, sl], srall[:, sl], cr[:])
            nb = bpool.tile([128, G], f32, tag="nb")
            nc.vector.tensor_mul(nb[:], npl[:, sl], bias[:])
            nc.vector.tensor_sub(srall[:, sl], srall[:, sl], nb[:])
            col += Wb

        # final combine on [128, SB] planes (f32), split for tail overlap
        tmpA = cpool.tile([128, SB], f32)
        half = SB // 2
        for hs in (slice(0, half), slice(half, SB)):
            nc.vector.scalar_tensor_tensor(
                tmpA[:, hs], mnall[:, hs], W1, apl[:, hs], OP.mult, OP.add)
            nc.vector.scalar_tensor_tensor(
                apl[:, hs], mxall[:, hs], W2, tmpA[:, hs], OP.mult, OP.add)
            nc.vector.scalar_tensor_tensor(
                tmpA[:, hs], srall[:, hs], W3, apl[:, hs], OP.mult, OP.add)
            nc.vector.scalar_tensor_tensor(
                apl[:, hs], sxall[:, hs], W4, tmpA[:, hs], OP.mult, OP.add)
            nc.sync.dma_start(odr[:, hs], apl[:, hs])

    nc.compile()
    return nc


# revision 9
# speedup vs baseline: 2.7380x; 1.0793x over previous
"""Trainium2 Bass kernel for AdaptiveReLU segment-reduce.

Reference computation (per segment s over instance rows x[i] with batch_idx[i]==s):
    mn = min, mx = max, sums = sum, n = count
    bias = t*mx + (1-t)*mn            (t clamped to [0,1], per feature)
    relu_sum = sum(relu(x - bias))
    out[s,f] = W0*n + W1*mn + W2*mx + W3*relu_sum + W4*sums

Strategy: host-side sort + count-sorted packing so every segment lives on one
core with a few % padding, then a fully local (collective-free) SPMD kernel on
8 NeuronCores.

Layout (per core):
  - Segments are globally sorted by count (desc).  Consecutive runs of 256*m
    segments per core (m positions merged into one "superblock") share one
    padded length L (equal across cores -> one SPMD graph).  A DP chooses the
    superblock partition + L to trade padding vs per-op overhead vs fold ops.
  - Superblock SBUF tile: [128 partitions, L*m*128 columns] bf16, partition
    p = par*64 + f (par in {0,1}, f = feature), column j*(m*128) + idx
    (j = row-within-segment, idx = segment-group).  Padding rows replicate
    the segment's first row, so min/max are unaffected; sums are corrected
    with the known pad counts.
  - Reductions are pairwise-halving trees of tensor_tensor ops (bf16 2x_1p,
    2 elem/cycle) instead of tensor_reduce (1x only).  min/max trees on bf16
    are exact; sum trees add ~0.2% noise, well under the 2e-2 gate.
  - relu_sum uses the max trick:  sum(relu(x-b)) = sum(max(x,b)) - n*b,
    with max(x, bias) computed in place over the x tile (no second buffer,
    no ScalarE dependency).
"""

import os
import numpy as np
import ml_dtypes

F = 64            # feature dim
G = 128           # segment-groups per position (2 parities x 64 features)
SPB = 2 * G       # segments per position per core
NCORES = 8
MAX_LM = 96       # SBUF cap: L * m <= MAX_LM  (tile = L*m*128 cols bf16)

BF16 = ml_dtypes.bfloat16


def _nfolds(L):
    n = 0
    while L > 1:
        if L % 2:
            n += 1
        L //= 2
    return n


def _partition(Ls):
    """DP partition of block positions into superblocks.

    Returns list of (start, m, Lpad).  Cost model (ns):
      padding: 270 per extra L-unit per position
      folds:   4 trees * (m*64 + 146) per odd level
      fixed:   5500 per superblock
    """
    NB = len(Ls)
    INF = float("inf")
    best = [INF] * (NB + 1)
    choice = [None] * (NB + 1)
    best[NB] = 0.0
    for i in range(NB - 1, -1, -1):
        for j in range(i + 1, NB + 1):
            m = j - i
            Lmax = int(Ls[i])
            if Lmax * m > MAX_LM:
                break
            c_best = INF
            lp_best = Lmax
            for Lp in range(Lmax, min(Lmax + 13, MAX_LM // m + 1)):
                pad = sum(Lp - int(Ls[k]) for k in range(i, j))
                c = pad * 270.0 + _nfolds(Lp) * 4 * (m * 64 + 146) + 5500.0
                if c < c_best:
                    c_best, lp_best = c, Lp
            if c_best + best[j] < best[i]:
                best[i] = c_best + best[j]
                choice[i] = (j, lp_best)
    out = []
    i = 0
    while i < NB:
        j, lp = choice[i]
        out.append((i, j - i, lp))
        i = j
    return out


def _pack(x, batch_idx, S, Wvals):
    """Sort+pack inputs. Returns (in_maps, sblocks, order)."""
    rps = SPB * NCORES                      # ranks per position
    NB = S // rps
    assert S % rps == 0, (S, rps)

    counts = np.bincount(batch_idx, minlength=S).astype(np.int64)
    order = np.argsort(-counts, kind="stable").astype(np.int64)
    sc = counts[order]
    assert sc[-1] >= 1, "empty segments unsupported"
    Ls = np.maximum(sc[::rps], 1).astype(np.int64)        # [NB]
    sblocks = _partition(Ls)

    perm = np.argsort(batch_idx, kind="stable").astype(np.int64)
    seg_start = np.zeros(S + 1, np.int64)
    np.cumsum(counts, out=seg_start[1:])

    W0 = float(Wvals[0])
    in_maps = [dict() for _ in range(NCORES)]
    W_total = int(sum(m * G * Lp for (_, m, Lp) in sblocks))
    xbf = x.astype(BF16)
    for c in range(NCORES):
        xcore = np.empty((128, W_total), BF16)
        aplane = np.empty((128, G * NB), np.float32)   # W0 * n
        pplane = np.empty((128, G * NB), BF16)         # pad count Lp - n
        col = 0
        for (b0, m, Lp) in sblocks:
            Gm = m * G
            # ranks for positions b0..b0+m-1, concatenated: [m*SPB]
            ranks = (rps * (b0 + np.arange(m))[:, None]
                     + SPB * c + np.arange(SPB)[None, :]).ravel()
            segs = order[ranks]                            # [m*256]
            cnt = counts[segs]
            j = np.arange(Lp)[None, :]
            jeff = np.where(j < cnt[:, None], j, 0)        # replicate first row
            rows = perm[seg_start[segs][:, None] + jeff]   # [m*256, Lp]
            blk = xbf[rows]                                # [m*256, Lp, 64]
            # (b_rel, g, par, j, f) -> (par, f, j, b_rel, g)
            blk = blk.reshape(m, G, 2, Lp, F).transpose(2, 4, 3, 0, 1)
            xcore[:, col:col + Lp * Gm] = blk.reshape(128, Lp * Gm)
            cblk = cnt.reshape(m * G, 2).T                 # [2, m*G]
            sl = slice(b0 * G, b0 * G + Gm)
            aplane[:, sl] = np.broadcast_to(
                (W0 * cblk)[:, None, :], (2, F, Gm)).reshape(128, Gm)
            pplane[:, sl] = np.broadcast_to(
                (float(Lp) - cblk)[:, None, :], (2, F, Gm)).reshape(128, Gm)
            col += Lp * Gm
        in_maps[c]["xb"] = xcore
        in_maps[c]["apl"] = aplane
        in_maps[c]["ppl"] = pplane
    return in_maps, sblocks, order


def _tree(nc, pool, src_ap, L, Gm, dst_ap, op, bf16):
    """Pairwise-halving reduction tree over j (column-groups of Gm)."""
    assert L >= 2
    cur = src_ap
    Lc = L
    lvl = 0
    while Lc > 1:
        h = Lc // 2
        odd = Lc % 2 == 1
        if h == 1:
            nxt = dst_ap          # final level writes the stats plane
        else:
            t = pool.tile([128, h * Gm], bf16, tag=f"tr{lvl}")
            nxt = t[:]
        nc.vector.tensor_tensor(
            nxt[:, 0:h * Gm], cur[:, 0:h * Gm], cur[:, h * Gm:2 * h * Gm],
            op=op)
        if odd:
            nc.vector.tensor_tensor(
                nxt[:, 0:Gm], nxt[:, 0:Gm], cur[:, 2 * h * Gm:Lc * Gm], op=op)
        cur = nxt
        Lc = h
        lvl += 1


LAST_EXEC_NS = None
LAST_RESULTS = None


def kernel(x, batch_idx, max_index, t, W):
    global LAST_EXEC_NS, LAST_RESULTS
    x = np.ascontiguousarray(np.asarray(x, dtype=np.float32))
    bidx = np.asarray(batch_idx).astype(np.int64)
    S = int(max_index)
    t_np = np.asarray(t, dtype=np.float32).reshape(F)
    W_np = np.asarray(W, dtype=np.float32).reshape(-1)
    assert x.shape[1] == F and W_np.shape[0] == 5

    in_maps, sblocks, order = _pack(x, bidx, S, W_np)
    NB = S // (SPB * NCORES)
    tpar = np.tile(t_np, 2).reshape(128, 1).astype(np.float32)
    for m in in_maps:
        m["tpar"] = tpar

    nc = _build(sblocks, NB, W_np)

    if os.environ.get("KERNEL_SIM", "0") == "1":
        from concourse.bass_interp import CoreSim
        outs = []
        for c in range(NCORES):
            sim = CoreSim(nc, trace=False)
            for k, v in in_maps[c].items():
                sim.tensor(k)[:] = v
            sim.simulate(check_with_hw=False)
            outs.append(np.array(sim.tensor("out")))
        results = [{"out": o} for o in outs]
        LAST_EXEC_NS = None
    else:
        from concourse import bass_utils
        trace = os.environ.get("KERNEL_TRACE", "0") == "1"
        tmpdir = os.environ.get("KERNEL_TRACE_DIR") or None
        res = bass_utils.run_bass_kernel_spmd(
            nc, in_maps, core_ids=list(range(NCORES)),
            trace=trace, tmpdir=tmpdir)
        results = res.results
        LAST_EXEC_NS = res.exec_time_ns
        LAST_RESULTS = res

    # Unpack: out_dev [128, G*NB] -> [S, F] in original segment order
    rps = SPB * NCORES
    out_full = np.empty((S, F), np.float32)
    for c in range(NCORES):
        od = np.asarray(results[c]["out"])              # [128, G*NB]
        v = od.reshape(2, F, NB, G).transpose(2, 3, 0, 1)   # [NB, G, 2, F]
        v = v.reshape(NB * SPB, F)                      # rank-chunk order
        ranks = (rps * np.arange(NB)[:, None] + SPB * c
                 + np.arange(SPB)[None, :]).ravel()
        out_full[order[ranks]] = v
    return out_full


def _build(sblocks, NB, Wvals):
    """Build the SPMD Bass graph. Returns compiled Bacc module."""
    import concourse.tile as tile
    from concourse import bacc, mybir

    f32 = mybir.dt.float32
    bf16 = mybir.dt.bfloat16
    OP = mybir.AluOpType

    SB = G * NB
    W_total = int(sum(m * G * Lp for (_, m, Lp) in sblocks))
    W0, W1, W2, W3, W4 = [float(v) for v in Wvals]

    nc = bacc.Bacc("TRN2", target_bir_lowering=False, debug=False,
                   num_devices=NCORES)
    xdr = nc.dram_tensor("xb", [128, W_total], bf16, kind="ExternalInput").ap()
    adr = nc.dram_tensor("apl", [128, SB], f32, kind="ExternalInput").ap()
    pdr = nc.dram_tensor("ppl", [128, SB], bf16, kind="ExternalInput").ap()
    tdr = nc.dram_tensor("tpar", [128, 1], f32, kind="ExternalInput").ap()
    odr = nc.dram_tensor("out", [128, SB], f32, kind="ExternalOutput").ap()

    with tile.TileContext(nc) as tc, \
         tc.tile_pool(name="xpool", bufs=2) as xpool, \
         tc.tile_pool(name="tpool", bufs=2) as tpool, \
         tc.tile_pool(name="bpool", bufs=2) as bpool, \
         tc.tile_pool(name="cpool", bufs=1) as cpool:

        tpp = cpool.tile([128, 1], f32)
        nc.sync.dma_start(tpp[:], tdr)
        tcl = cpool.tile([128, 1], f32)
        nc.vector.tensor_scalar(tcl[:], tpp[:], 0.0, 1.0, OP.max, OP.min)
        onemt = cpool.tile([128, 1], f32)
        nc.vector.tensor_scalar(onemt[:], tcl[:], -1.0, 1.0, OP.mult, OP.add)

        apl = cpool.tile([128, SB], f32)
        nc.sync.dma_start(apl[:], adr)
        ppl = cpool.tile([128, SB], bf16)
        nc.sync.dma_start(ppl[:], pdr)

        # persistent stats planes (min/max are exact in bf16)
        mnall = cpool.tile([128, SB], bf16)
        mxall = cpool.tile([128, SB], bf16)
        sxall = cpool.tile([128, SB], f32)
        srall = cpool.tile([128, SB], f32)

        col = 0
        for (b0, m, Lp) in sblocks:
            Gm = m * G
            sl = slice(b0 * G, b0 * G + Gm)
            Wb = Lp * Gm
            xt = xpool.tile([128, Wb], bf16, tag="xt")
            nc.sync.dma_start(xt[:], xdr[:, col:col + Wb])

            _tree(nc, tpool, xt[:], Lp, Gm, mnall[:, sl], OP.min, bf16)
            _tree(nc, tpool, xt[:], Lp, Gm, mxall[:, sl], OP.max, bf16)
            _tree(nc, tpool, xt[:], Lp, Gm, sxall[:, sl], OP.add, bf16)

            # pad correction for sum(x) (uses original first row, pre-max)
            cx = bpool.tile([128, Gm], bf16, tag="cx")
            nc.vector.tensor_mul(cx[:], ppl[:, sl], xt[:, 0:Gm])
            nc.vector.tensor_sub(sxall[:, sl], sxall[:, sl], cx[:])

            # bias = t*mx + (1-t)*mn  (bf16)
            biasA = bpool.tile([128, Gm], bf16, tag="biasA")
            nc.vector.tensor_scalar_mul(biasA[:], mxall[:, sl], tcl[:])
            bias = bpool.tile([128, Gm], bf16, tag="bias")
            nc.vector.scalar_tensor_tensor(
                bias[:], mnall[:, sl], onemt[:], biasA[:], OP.mult, OP.add)

            # max trick, in place: xt <- max(xt, bias)
            xjg = xt[:].rearrange("p (j g) -> p j g", g=Gm)
            bias_b = bias[:].unsqueeze(1).broadcast_to([128, Lp, Gm])
            nc.vector.tensor_tensor(xjg, xjg, bias_b, op=OP.max)

            _tree(nc, tpool, xt[:], Lp, Gm, srall[:, sl], OP.add, bf16)

            # relu_sum corrections: sm -= (Lp-n)*mt0 + n*bias
            cr = bpool.tile([128, Gm], bf16, tag="cr")
            nc.vector.tensor_mul(cr[:], ppl[:, sl], xt[:, 0:Gm])
            nc.vector.tensor_sub(srall[:, sl], srall[:, sl], cr[:])
            pn = bpool.tile([128, Gm], bf16, tag="pn")
            nc.vector.tensor_scalar(pn[:], ppl[:, sl], -1.0, float(Lp),
                                    OP.mult, OP.add)
            nb = bpool.tile([128, Gm], f32, tag="nb")
            nc.vector.tensor_mul(nb[:], pn[:], bias[:])
            nc.vector.tensor_sub(srall[:, sl], srall[:, sl], nb[:])
            col += Wb

        # final combine on [128, SB] planes (f32), split for tail overlap
        tmpA = cpool.tile([128, SB], f32)
        half = SB // 2
        for hs in (slice(0, half), slice(half, SB)):
            nc.vector.scalar_tensor_tensor(
                tmpA[:, hs], mnall[:, hs], W1, apl[:, hs], OP.mult, OP.add)
            nc.vector.scalar_tensor_tensor(
                apl[:, hs], mxall[:, hs], W2, tmpA[:, hs], OP.mult, OP.add)
            nc.vector.scalar_tensor_tensor(
                tmpA[:, hs], srall[:, hs], W3, apl[:, hs], OP.mult, OP.add)
            nc.vector.scalar_tensor_tensor(
                apl[:, hs], sxall[:, hs], W4, tmpA[:, hs], OP.mult, OP.add)
            nc.sync.dma_start(odr[:, hs], apl[:, hs])

    nc.compile()
    return nc


# revision 15
# speedup vs baseline: 2.7605x; 1.0082x over previous
"""Trainium2 Bass kernel for AdaptiveReLU segment-reduce.

Reference computation (per segment s over instance rows x[i] with batch_idx[i]==s):
    mn = min, mx = max, sums = sum, n = count
    bias = t*mx + (1-t)*mn            (t clamped to [0,1], per feature)
    relu_sum = sum(relu(x - bias))
    out[s,f] = W0*n + W1*mn + W2*mx + W3*relu_sum + W4*sums

Strategy: host-side sort + count-sorted packing so every segment lives on one
core with a few % padding, then a fully local (collective-free) SPMD kernel on
8 NeuronCores.

Layout (per core):
  - Segments are globally sorted by count (desc).  Consecutive runs of 256*m
    segments per core (m positions merged into one "superblock") share one
    padded length L (equal across cores -> one SPMD graph).  A DP chooses the
    superblock partition + L to trade padding vs per-op overhead vs fold ops.
  - Superblock SBUF tile: [128 partitions, L*m*128 columns] bf16, partition
    p = par*64 + f (par in {0,1}, f = feature), column j*(m*128) + idx
    (j = row-within-segment, idx = segment-group).  Padding rows replicate
    the segment's first row, so min/max are unaffected; sums are corrected
    with the known pad counts.
  - Reductions are pairwise-halving trees of tensor_tensor ops (bf16 2x_1p,
    2 elem/cycle) instead of tensor_reduce (1x only).  min/max trees on bf16
    are exact; sum trees add ~0.2% noise, well under the 2e-2 gate.
  - relu_sum uses the max trick:  sum(relu(x-b)) = sum(max(x,b)) - n*b,
    with max(x, bias) computed in place over the x tile (no second buffer,
    no ScalarE dependency).
"""

import os
import numpy as np
import ml_dtypes

F = 64            # feature dim
G = 128           # segment-groups per position (2 parities x 64 features)
SPB = 2 * G       # segments per position per core
NCORES = 8
MAX_LM = 96       # SBUF cap: L * m <= MAX_LM  (tile = L*m*128 cols bf16)

BF16 = ml_dtypes.bfloat16


def _nfolds(L):
    n = 0
    while L > 1:
        if L % 2:
            n += 1
        L //= 2
    return n


def _partition(Ls):
    """DP partition of block positions into superblocks.

    Returns list of (start, m, Lpad).  Cost model (ns):
      padding: 270 per extra L-unit per position
      folds:   4 trees * (m*64 + 146) per odd level
      fixed:   5500 per superblock
    """
    NB = len(Ls)
    INF = float("inf")
    best = [INF] * (NB + 1)
    choice = [None] * (NB + 1)
    best[NB] = 0.0
    for i in range(NB - 1, -1, -1):
        for j in range(i + 1, NB + 1):
            m = j - i
            Lmax = int(Ls[i])
            if Lmax * m > MAX_LM:
                break
            c_best = INF
            lp_best = Lmax
            for Lp in range(Lmax, min(Lmax + 13, MAX_LM // m + 1)):
                pad = sum(Lp - int(Ls[k]) for k in range(i, j))
                c = pad * 270.0 + _nfolds(Lp) * 4 * (m * 64 + 146) + 5500.0
                if c < c_best:
                    c_best, lp_best = c, Lp
            if c_best + best[j] < best[i]:
                best[i] = c_best + best[j]
                choice[i] = (j, lp_best)
    out = []
    i = 0
    while i < NB:
        j, lp = choice[i]
        out.append((i, j - i, lp))
        i = j
    return out


def _pack(x, batch_idx, S, Wvals):
    """Sort+pack inputs. Returns (in_maps, sblocks, order)."""
    rps = SPB * NCORES                      # ranks per position
    NB = S // rps
    assert S % rps == 0, (S, rps)

    counts = np.bincount(batch_idx, minlength=S).astype(np.int64)
    order = np.argsort(-counts, kind="stable").astype(np.int64)
    sc = counts[order]
    assert sc[-1] >= 1, "empty segments unsupported"
    Ls = np.maximum(sc[::rps], 1).astype(np.int64)        # [NB]
    sblocks = _partition(Ls)

    perm = np.argsort(batch_idx, kind="stable").astype(np.int64)
    seg_start = np.zeros(S + 1, np.int64)
    np.cumsum(counts, out=seg_start[1:])

    W0, W4 = float(Wvals[0]), float(Wvals[4])
    in_maps = [dict() for _ in range(NCORES)]
    W_total = int(sum(m * G * Lp for (_, m, Lp) in sblocks))
    xbf = x.astype(BF16)
    for c in range(NCORES):
        xcore = np.empty((128, W_total), BF16)
        aplane = np.empty((128, G * NB), np.float32)   # W0*n - W4*(Lp-n)*x0
        pplane = np.empty((128, G * NB), BF16)         # pad count Lp - n
        nplane = np.empty((128, G * NB), BF16)         # count n
        col = 0
        for (b0, m, Lp) in sblocks:
            Gm = m * G
            # ranks for positions b0..b0+m-1, concatenated: [m*SPB]
            ranks = (rps * (b0 + np.arange(m))[:, None]
                     + SPB * c + np.arange(SPB)[None, :]).ravel()
            segs = order[ranks]                            # [m*256]
            cnt = counts[segs]
            j = np.arange(Lp)[None, :]
            jeff = np.where(j < cnt[:, None], j, 0)        # replicate first row
            rows = perm[seg_start[segs][:, None] + jeff]   # [m*256, Lp]
            blk = xbf[rows]                                # [m*256, Lp, 64]
            # (b_rel, g, par, j, f) -> (par, f, j, b_rel, g)
            blk = blk.reshape(m, G, 2, Lp, F).transpose(2, 4, 3, 0, 1)
            xcore[:, col:col + Lp * Gm] = blk.reshape(128, Lp * Gm)
            cblk = cnt.reshape(m * G, 2).T                 # [2, m*G]
            sl = slice(b0 * G, b0 * G + Gm)
            # x0 in device layout [2, F, Gm]: value of the packed row j=0
            # (bf16-rounded, matching what the device sum tree adds for pads)
            x0dev = np.asarray(
                blk[:, :, 0, :, :].reshape(128, Gm), np.float32)
            pads = np.broadcast_to(
                (float(Lp) - cblk)[:, None, :], (2, F, Gm)).reshape(128, Gm)
            aplane[:, sl] = (np.broadcast_to(
                (W0 * cblk)[:, None, :], (2, F, Gm)).reshape(128, Gm)
                - W4 * pads * x0dev)
            pplane[:, sl] = pads
            nplane[:, sl] = np.broadcast_to(
                cblk[:, None, :], (2, F, Gm)).reshape(128, Gm)
            col += Lp * Gm
        in_maps[c]["xb"] = xcore
        in_maps[c]["apl"] = aplane
        in_maps[c]["ppl"] = pplane
        in_maps[c]["npl"] = nplane
    return in_maps, sblocks, order


def _tree(nc, pool, src_ap, L, Gm, dst_ap, op, bf16):
    """Pairwise-halving reduction tree over j (column-groups of Gm)."""
    assert L >= 2
    cur = src_ap
    Lc = L
    lvl = 0
    while Lc > 1:
        h = Lc // 2
        odd = Lc % 2 == 1
        if h == 1:
            nxt = dst_ap          # final level writes the stats plane
        else:
            t = pool.tile([128, h * Gm], bf16, tag=f"tr{lvl}")
            nxt = t[:]
        nc.vector.tensor_tensor(
            nxt[:, 0:h * Gm], cur[:, 0:h * Gm], cur[:, h * Gm:2 * h * Gm],
            op=op)
        if odd:
            nc.vector.tensor_tensor(
                nxt[:, 0:Gm], nxt[:, 0:Gm], cur[:, 2 * h * Gm:Lc * Gm], op=op)
        cur = nxt
        Lc = h
        lvl += 1


LAST_EXEC_NS = None
LAST_RESULTS = None


def kernel(x, batch_idx, max_index, t, W):
    global LAST_EXEC_NS, LAST_RESULTS
    x = np.ascontiguousarray(np.asarray(x, dtype=np.float32))
    bidx = np.asarray(batch_idx).astype(np.int64)
    S = int(max_index)
    t_np = np.asarray(t, dtype=np.float32).reshape(F)
    W_np = np.asarray(W, dtype=np.float32).reshape(-1)
    assert x.shape[1] == F and W_np.shape[0] == 5

    in_maps, sblocks, order = _pack(x, bidx, S, W_np)
    NB = S // (SPB * NCORES)
    tpar = np.tile(t_np, 2).reshape(128, 1).astype(np.float32)
    for m in in_maps:
        m["tpar"] = tpar

    nc = _build(sblocks, NB, W_np)

    if os.environ.get("KERNEL_SIM", "0") == "1":
        from concourse.bass_interp import CoreSim
        outs = []
        for c in range(NCORES):
            sim = CoreSim(nc, trace=False)
            for k, v in in_maps[c].items():
                sim.tensor(k)[:] = v
            sim.simulate(check_with_hw=False)
            outs.append(np.array(sim.tensor("out")))
        results = [{"out": o} for o in outs]
        LAST_EXEC_NS = None
    else:
        from concourse import bass_utils
        trace = os.environ.get("KERNEL_TRACE", "0") == "1"
        tmpdir = os.environ.get("KERNEL_TRACE_DIR") or None
        res = bass_utils.run_bass_kernel_spmd(
            nc, in_maps, core_ids=list(range(NCORES)),
            trace=trace, tmpdir=tmpdir)
        results = res.results
        LAST_EXEC_NS = res.exec_time_ns
        LAST_RESULTS = res

    # Unpack: out_dev [128, G*NB] -> [S, F] in original segment order
    rps = SPB * NCORES
    out_full = np.empty((S, F), np.float32)
    for c in range(NCORES):
        od = np.asarray(results[c]["out"])              # [128, G*NB]
        v = od.reshape(2, F, NB, G).transpose(2, 3, 0, 1)   # [NB, G, 2, F]
        v = v.reshape(NB * SPB, F)                      # rank-chunk order
        ranks = (rps * np.arange(NB)[:, None] + SPB * c
                 + np.arange(SPB)[None, :]).ravel()
        out_full[order[ranks]] = v
    return out_full


def _build(sblocks, NB, Wvals):
    """Build the SPMD Bass graph. Returns compiled Bacc module."""
    import concourse.tile as tile
    from concourse import bacc, mybir

    f32 = mybir.dt.float32
    bf16 = mybir.dt.bfloat16
    OP = mybir.AluOpType

    SB = G * NB
    W_total = int(sum(m * G * Lp for (_, m, Lp) in sblocks))
    W0, W1, W2, W3, W4 = [float(v) for v in Wvals]

    nc = bacc.Bacc("TRN2", target_bir_lowering=False, debug=False,
                   num_devices=NCORES)
    xdr = nc.dram_tensor("xb", [128, W_total], bf16, kind="ExternalInput").ap()
    adr = nc.dram_tensor("apl", [128, SB], f32, kind="ExternalInput").ap()
    pdr = nc.dram_tensor("ppl", [128, SB], bf16, kind="ExternalInput").ap()
    ndr = nc.dram_tensor("npl", [128, SB], bf16, kind="ExternalInput").ap()
    tdr = nc.dram_tensor("tpar", [128, 1], f32, kind="ExternalInput").ap()
    odr = nc.dram_tensor("out", [128, SB], f32, kind="ExternalOutput").ap()

    with tile.TileContext(nc) as tc, \
         tc.tile_pool(name="xpool", bufs=2) as xpool, \
         tc.tile_pool(name="tpool", bufs=1) as tpool, \
         tc.tile_pool(name="bpool", bufs=2) as bpool, \
         tc.tile_pool(name="cpool", bufs=1) as cpool:

        tpp = cpool.tile([128, 1], f32)
        nc.sync.dma_start(tpp[:], tdr)
        tcl = cpool.tile([128, 1], f32)
        nc.vector.tensor_scalar(tcl[:], tpp[:], 0.0, 1.0, OP.max, OP.min)
        onemt = cpool.tile([128, 1], f32)
        nc.vector.tensor_scalar(onemt[:], tcl[:], -1.0, 1.0, OP.mult, OP.add)

        apl = cpool.tile([128, SB], f32)
        nc.sync.dma_start(apl[:], adr)
        ppl = cpool.tile([128, SB], bf16)
        nc.sync.dma_start(ppl[:], pdr)
        npl = cpool.tile([128, SB], bf16)
        nc.sync.dma_start(npl[:], ndr)

        # persistent stats planes (min/max are exact in bf16)
        mnall = cpool.tile([128, SB], bf16)
        mxall = cpool.tile([128, SB], bf16)
        sxall = cpool.tile([128, SB], f32)
        srall = cpool.tile([128, SB], f32)

        col = 0
        for (b0, m, Lp) in sblocks:
            Gm = m * G
            sl = slice(b0 * G, b0 * G + Gm)
            Wb = Lp * Gm
            xt = xpool.tile([128, Wb], bf16, tag="xt")
            # split the load across DMA queues
            qc = (Wb + 3) // 4
            qc += qc % 2            # keep 4-byte alignment of chunk starts
            for q0 in range(0, Wb, qc):
                q1 = min(q0 + qc, Wb)
                nc.sync.dma_start(xt[:, q0:q1], xdr[:, col + q0:col + q1])

            _tree(nc, tpool, xt[:], Lp, Gm, mnall[:, sl], OP.min, bf16)
            _tree(nc, tpool, xt[:], Lp, Gm, mxall[:, sl], OP.max, bf16)
            _tree(nc, tpool, xt[:], Lp, Gm, sxall[:, sl], OP.add, bf16)
            # (sum-x pad correction is folded into the host apl plane)

            # bias = t*mx + (1-t)*mn  (bf16)
            biasA = bpool.tile([128, Gm], bf16, tag="biasA")
            nc.vector.tensor_scalar_mul(biasA[:], mxall[:, sl], tcl[:])
            bias = bpool.tile([128, Gm], bf16, tag="bias")
            nc.vector.scalar_tensor_tensor(
                bias[:], mnall[:, sl], onemt[:], biasA[:], OP.mult, OP.add)

            # max trick, in place: xt <- max(xt, bias)
            xjg = xt[:].rearrange("p (j g) -> p j g", g=Gm)
            bias_b = bias[:].unsqueeze(1).broadcast_to([128, Lp, Gm])
            nc.vector.tensor_tensor(xjg, xjg, bias_b, op=OP.max)

            _tree(nc, tpool, xt[:], Lp, Gm, srall[:, sl], OP.add, bf16)

            # relu_sum corrections: sm -= (Lp-n)*mt0 + n*bias
            cr = bpool.tile([128, Gm], bf16, tag="cr")
            nc.vector.tensor_mul(cr[:], ppl[:, sl], xt[:, 0:Gm])
            nc.vector.tensor_sub(srall[:, sl], srall[:, sl], cr[:])
            nb = bpool.tile([128, Gm], f32, tag="nb")
            nc.vector.tensor_mul(nb[:], npl[:, sl], bias[:])
            nc.vector.tensor_sub(srall[:, sl], srall[:, sl], nb[:])
            col += Wb

        # final combine on [128, SB] planes (f32), quarter-chunked so the
        # out-DMA of chunk q overlaps the stt chain of chunk q+1
        tmpA = cpool.tile([128, SB], f32)
        qn = SB // 4
        for q in range(4):
            qs = slice(q * qn, (q + 1) * qn)
            obuf = bpool.tile([128, qn], f32, tag="obuf")
            nc.vector.scalar_tensor_tensor(
                tmpA[:, qs], mnall[:, qs], W1, apl[:, qs], OP.mult, OP.add)
            nc.vector.scalar_tensor_tensor(
                apl[:, qs], mxall[:, qs], W2, tmpA[:, qs], OP.mult, OP.add)
            nc.vector.scalar_tensor_tensor(
                tmpA[:, qs], srall[:, qs], W3, apl[:, qs], OP.mult, OP.add)
            nc.vector.scalar_tensor_tensor(
                obuf[:], sxall[:, qs], W4, tmpA[:, qs], OP.mult, OP.add)
            nc.sync.dma_start(odr[:, qs], obuf[:])

    nc.compile()
    return nc
